# revision 6
# baseline (speedup 1.0000x reference)
"""ChebNet (gnn_message_passing) Trainium2 kernel — 8 NeuronCores, SPMD.

Strategy:
- Nodes row-sharded 12500/core (edge_row is sorted). Each core's rows are
  pre-assigned to 4 fixed "parts" (by original index), padded to 3200 rows,
  and permuted within-part by a window-packing permutation.
- Per Chebyshev hop: AllGather of the [3200,64] part tables builds each
  core's full gather table [102400, 64]; the 4 parts double as the int16
  source buckets for dma_gather AND as the 4 chunked AllGathers.
- spmm = dma_gather (256B rows) + selector matmuls: chunk = 128 gathered
  rows (lhsT [128 slots, 64]) x selector rhs [128, 13] -> PSUM [64, 13-row
  window slice], accumulated across the 4 buckets; transposed back to
  row-major via TensorE; T_{k+1} = 2*S - T_{k-1} with the 2x folded into
  the hop>=2 selector tables and T_{k-1} streamed from DRAM.
- Encoder MLP runs transposed (h1T = W1c.T @ xT) so biases are
  per-partition; x is transposed/permuted on host.
- z = sum_k gamma_k T_k computed in a final batched pass over the 9 stored
  T tables.
"""
import numpy as np

# ---------------- configuration (hardcoded for this problem) ----------------
IN_DIM, HID, D = 512, 256, 64
KHOPS = 8
BLK = 128
W = 13                      # window width (psum free-dim slice per chunk)
NW = 10                     # chunks per (block, bucket)
WLAST = BLK - W * (NW - 1)  # 11

FULL = dict(N=100000, NC=8, NPART=4, PART_REAL=3125, PART_PAD=3200, BPC=5,
            KHOPS=8)


def _derived(cfg):
    c = dict(cfg)
    c["RPC"] = c["N"] // c["NC"]
    c["R"] = c["NPART"] * c["PART_PAD"]
    c["NBLK"] = c["R"] // BLK
    c["BUCKET"] = c["NC"] * c["PART_PAD"]
    c["G"] = c["NPART"] * c["BUCKET"]
    c["NGRP"] = c["NBLK"] // c["BPC"]
    c["IPC"] = c["BPC"] * NW * 128          # idxs per gather call
    return c


# ---------------------------- host preprocessing ----------------------------

def _pack_windows(deg_vec, part_pad):
    """deg_vec [PART_REAL, 4] -> within-part permuted position for each row,
    s.t. every (13-row window, bucket) degree sum <= 128."""
    nwin = (part_pad // BLK) * NW
    win_rows = np.array([WLAST if (w % NW) == NW - 1 else W for w in range(nwin)])
    loads = np.zeros((nwin, 4), np.int64)
    used = np.zeros(nwin, np.int64)
    order = np.argsort(-deg_vec.sum(1), kind="stable")
    win_of = np.empty(len(deg_vec), np.int64)
    for r in order:
        d = deg_vec[r]
        ok = (used < win_rows) & ((loads + d) <= 128).all(1)
        if not ok.any():
            raise RuntimeError("window packing infeasible; increase NW")
        cand = np.where(ok)[0]
        w = cand[np.argmin((loads[cand] + d).max(1))]
        win_of[r] = w
        loads[w] += d
        used[w] += 1
    j = np.zeros(len(deg_vec), np.int64)
    fill = np.zeros(nwin, np.int64)
    for r in order:
        w = win_of[r]
        j[r] = (w // NW) * BLK + (w % NW) * W + fill[w]
        fill[w] += 1
    return j


def _preprocess(edge_row, edge_col, edge_vals, c):
    NC, RPC, PART_REAL = c["NC"], c["RPC"], c["PART_REAL"]
    PART_PAD, R, NBLK = c["PART_PAD"], c["R"], c["NBLK"]
    BUCKET, NGRP, BPC, IPC = c["BUCKET"], c["NGRP"], c["BPC"], c["IPC"]
    N = c["N"]

    edge_row = np.asarray(edge_row).astype(np.int64)
    edge_col = np.asarray(edge_col).astype(np.int64)
    edge_vals = np.asarray(edge_vals).astype(np.float32)
    src_part = (edge_col % RPC) // PART_REAL

    lo = np.searchsorted(edge_row, np.arange(NC) * RPC)
    hi = np.searchsorted(edge_row, (np.arange(NC) + 1) * RPC)
    j_all = np.zeros((NC, c["NPART"], PART_REAL), np.int64)
    for m in range(NC):
        rl = edge_row[lo[m]:hi[m]] - m * RPC
        q = src_part[lo[m]:hi[m]]
        dv = np.zeros((RPC, 4), np.int64)
        np.add.at(dv, (rl, q), 1)
        for p in range(c["NPART"]):
            j_all[m, p] = _pack_windows(dv[p * PART_REAL:(p + 1) * PART_REAL],
                                        PART_PAD)

    v = np.arange(N)
    m_of, local = v // RPC, v % RPC
    p_of, o_of = local // PART_REAL, local % PART_REAL
    pos = p_of * BUCKET + m_of * PART_PAD + j_all[m_of, p_of, o_of]

    perm = np.full((NC, R), -1, np.int64)
    r_in_core = p_of * PART_PAD + j_all[m_of, p_of, o_of]
    perm[m_of, r_in_core] = local

    cores = []
    for m in range(NC):
        rl = edge_row[lo[m]:hi[m]] - m * RPC
        q = src_part[lo[m]:hi[m]]
        vals = edge_vals[lo[m]:hi[m]]
        boff = pos[edge_col[lo[m]:hi[m]]] - q * BUCKET
        gl = m * RPC + rl
        rr_core = p_of[gl] * PART_PAD + j_all[m, p_of[gl], o_of[gl]]
        b = rr_core // BLK
        rr = rr_core % BLK
        wi = np.minimum(rr // W, NW - 1)
        row_in_win = rr - wi * W

        chunk = (q * NBLK + b) * NW + wi          # bucket-major global chunk
        order = np.argsort(chunk, kind="stable")
        cs = chunk[order]
        slot = np.arange(len(cs)) - np.searchsorted(cs, cs)
        assert slot.max() < 128, "chunk overflow"

        nch = 4 * NBLK * NW
        idx_full = np.zeros((nch, 128), np.int64)
        sel_full = np.zeros((nch, 128, W), np.float32)
        idx_full[cs, slot] = boff[order]
        sel_full[cs, slot, row_in_win[order]] = vals[order]
        nfill = np.zeros(nch, np.int64)
        np.add.at(nfill, cs, 1)
        padm = np.arange(128)[None, :] >= nfill[:, None]
        idx_full[padm] = np.broadcast_to(idx_full[:, :1], idx_full.shape)[padm]

        # regroup chunks: device order is (q, grp, bl, wi); chunk id above is
        # (q, blk, wi) with blk = grp*BPC+bl -> already linear in that order.
        ncalls = 4 * NGRP
        idx_call = idx_full.reshape(ncalls, IPC)
        assert idx_call.max() < 2 ** 15
        i16 = idx_call.reshape(ncalls, IPC // 16, 16)
        idx16 = np.tile(i16.transpose(0, 2, 1), (1, 8, 1))  # [ncalls,128,IPC//16]
        idx16 = np.ascontiguousarray(idx16).astype(np.int16)

        sel = sel_full.reshape(ncalls, BPC * NW, 128, W)
        sel = np.ascontiguousarray(sel.transpose(0, 2, 1, 3)).reshape(
            ncalls, 128, BPC * NW * W)
        cores.append({"idx16": idx16.reshape(4, NGRP, 128, IPC // 16),
                      "sel1": sel.reshape(4, NGRP, 128, BPC * NW * W),
                      "sel2": (2.0 * sel).reshape(4, NGRP, 128, BPC * NW * W)})
    return {"pos": pos, "perm": perm, "cores": cores}


# ------------------------------ device program ------------------------------

def _build_program(c, gamma):
    import concourse.bacc as bacc
    import concourse.mybir as mybir
    from concourse.tile import TileContext
    from concourse.masks import make_identity

    f32, i16 = mybir.dt.float32, mybir.dt.int16
    NPART, R, NBLK, BUCKET, G = c["NPART"], c["R"], c["NBLK"], c["BUCKET"], c["G"]
    NGRP, BPC, IPC, K = c["NGRP"], c["BPC"], c["IPC"], c["KHOPS"]
    PART_PAD = c["PART_PAD"]
    RT = 512                               # MLP row-tile
    NRT = R // RT
    FLAT = R * D // 128                    # flat free-dim per partition

    nc = bacc.Bacc("TRN2", target_bir_lowering=False, num_devices=c["NC"])
    rg = [list(range(c["NC"]))]

    xT = nc.dram_tensor("xT", [IN_DIM, R], f32, kind="ExternalInput")
    W1 = nc.dram_tensor("W1", [IN_DIM, HID], f32, kind="ExternalInput")
    W2 = nc.dram_tensor("W2", [HID, D], f32, kind="ExternalInput")
    b1p = nc.dram_tensor("b1p", [128, HID // 128], f32, kind="ExternalInput")
    b2p = nc.dram_tensor("b2p", [D, 1], f32, kind="ExternalInput")
    idxT = nc.dram_tensor("idxT", [4, NGRP, 128, IPC // 16], i16, kind="ExternalInput")
    sel1 = nc.dram_tensor("sel1", [4, NGRP, 128, BPC * NW * W], f32, kind="ExternalInput")
    sel2 = nc.dram_tensor("sel2", [4, NGRP, 128, BPC * NW * W], f32, kind="ExternalInput")
    z_out = nc.dram_tensor("z_out", [128, FLAT], f32, kind="ExternalOutput")

    with TileContext(nc) as tc:
        with tc.tile_pool(name="dram", bufs=1, space="DRAM") as dp, \
             tc.tile_pool(name="const", bufs=1) as cp:
            t_own = [dp.tile([R, D], f32, tag=f"town{k}", name=f"town{k}")
                     for k in range(K + 1)]
            t_full = [[dp.tile([BUCKET, D], f32, tag=f"tf{i}_{q}",
                               name=f"tf{i}_{q}")
                       for q in range(4)] for i in range(2)]
            ident = cp.tile([64, 64], f32)
            make_identity(nc, ident)

            # ---------------- MLP (transposed) ----------------
            with tc.tile_pool(name="mlp", bufs=2) as mp, \
                 tc.tile_pool(name="mcp", bufs=1) as mcp, \
                 tc.tile_pool(name="mpp", bufs=2, space="PSUM") as mpp, \
                 tc.tile_pool(name="mpp2", bufs=2, space="PSUM") as mpp2:
                w1_sb = mcp.tile([128, (IN_DIM // 128) * HID], f32)
                for kc in range(IN_DIM // 128):
                    nc.sync.dma_start(w1_sb[:, kc * HID:(kc + 1) * HID],
                                      W1[kc * 128:(kc + 1) * 128, :])
                w2_sb = mcp.tile([128, (HID // 128) * D], f32)
                for kc in range(HID // 128):
                    nc.sync.dma_start(w2_sb[:, kc * D:(kc + 1) * D],
                                      W2[kc * 128:(kc + 1) * 128, :])
                b1_sb = mcp.tile([128, HID // 128], f32)
                nc.sync.dma_start(b1_sb[:], b1p[:])
                b2_sb = mcp.tile([D, 1], f32)
                nc.sync.dma_start(b2_sb[:], b2p[:])

                parts_done = 0
                for rt in range(NRT):
                    xt = mp.tile([128, (IN_DIM // 128) * RT], f32, tag="xt")
                    for kc in range(IN_DIM // 128):
                        nc.sync.dma_start(
                            xt[:, kc * RT:(kc + 1) * RT],
                            xT[kc * 128:(kc + 1) * 128, rt * RT:(rt + 1) * RT])
                    h1 = mp.tile([128, (HID // 128) * RT], f32, tag="h1")
                    for h in range(HID // 128):
                        ps1 = mpp.tile([128, RT], f32, space="PSUM", tag="ps1")
                        for kc in range(IN_DIM // 128):
                            nc.tensor.matmul(
                                ps1[:],
                                lhsT=w1_sb[:, kc * HID + h * 128:
                                           kc * HID + (h + 1) * 128],
                                rhs=xt[:, kc * RT:(kc + 1) * RT],
                                start=(kc == 0), stop=(kc == IN_DIM // 128 - 1))
                        nc.scalar.activation(
                            out=h1[:, h * RT:(h + 1) * RT], in_=ps1[:],
                            func=mybir.ActivationFunctionType.Relu,
                            bias=b1_sb[:, h:h + 1], scale=1.0)
                    ps2 = mpp.tile([D, RT], f32, space="PSUM", tag="ps2")
                    for kc in range(HID // 128):
                        nc.tensor.matmul(ps2[:],
                                         lhsT=w2_sb[:, kc * D:(kc + 1) * D],
                                         rhs=h1[:, kc * RT:(kc + 1) * RT],
                                         start=(kc == 0),
                                         stop=(kc == HID // 128 - 1))
                    hT = mp.tile([D, RT], f32, tag="hT")
                    nc.scalar.activation(out=hT[:], in_=ps2[:],
                                         func=mybir.ActivationFunctionType.Identity,
                                         bias=b2_sb[:, 0:1], scale=1.0)
                    for sb in range(RT // 128):
                        ps3 = mpp2.tile([128, D], f32, space="PSUM", tag="ps3")
                        nc.tensor.transpose(out=ps3[:],
                                            in_=hT[:, sb * 128:(sb + 1) * 128],
                                            identity=ident[:])
                        row = mp.tile([128, D], f32, tag="mrow")
                        nc.scalar.copy(out=row[:], in_=ps3[:])
                        blk = rt * (RT // 128) + sb
                        nc.sync.dma_start(
                            t_own[0][blk * 128:(blk + 1) * 128, :], row[:])
                    done = ((rt + 1) * RT) // PART_PAD
                    while parts_done < done:
                        q = parts_done
                        nc.gpsimd.collective_compute(
                            "AllGather", mybir.AluOpType.bypass,
                            replica_groups=rg,
                            ins=[t_own[0][q * PART_PAD:(q + 1) * PART_PAD, :]],
                            outs=[t_full[0][q][:]])  # hop-0 tables in set 0
                        parts_done += 1

            # ---------------- Chebyshev hops ----------------
            for k in range(1, K + 1):
                seld = sel1 if k == 1 else sel2
                src = t_full[(k - 1) % 2]
                with tc.tile_pool(name=f"h{k}", bufs=2) as hp, \
                     tc.tile_pool(name=f"hpp{k}", bufs=3, space="PSUM") as hpp, \
                     tc.tile_pool(name=f"hpp2{k}", bufs=3, space="PSUM") as hpp2:
                    parts_done = 0
                    for grp in range(NGRP):
                        gts, sts = [], []
                        for q in range(4):
                            it = hp.tile([128, IPC // 16], i16, tag=f"idx{q}")
                            nc.sync.dma_start(it[:], idxT[q, grp])
                            st = hp.tile([128, BPC * NW * W], f32, tag=f"sel{q}")
                            nc.sync.dma_start(st[:], seld[q, grp])
                            gt = hp.tile([128, BPC * NW * D], f32, tag=f"g{q}")
                            nc.gpsimd.dma_gather(
                                out_ap=gt[:].rearrange("p (c d) -> p c d", d=D),
                                in_ap=src[q][:],
                                idxs_ap=it[:],
                                num_idxs=IPC, num_idxs_reg=IPC, elem_size=D,
                                single_packet=False)
                            gts.append(gt)
                            sts.append(st)
                        for bl in range(BPC):
                            blk = grp * BPC + bl
                            ps = hpp.tile([64, 128], f32, space="PSUM", tag="ps")
                            for wi in range(NW):
                                lc = bl * NW + wi
                                n = WLAST if wi == NW - 1 else W
                                for q in range(4):
                                    nc.tensor.matmul(
                                        ps[:, wi * W:wi * W + n],
                                        lhsT=gts[q][:, lc * D:(lc + 1) * D],
                                        rhs=sts[q][:, lc * W:lc * W + n],
                                        start=(q == 0), stop=(q == 3))
                            sT = hp.tile([64, 128], f32, tag="sT")
                            nc.scalar.copy(out=sT[:], in_=ps[:])
                            ps2 = hpp2.tile([128, D], f32, space="PSUM", tag="ps2")
                            nc.tensor.transpose(out=ps2[:], in_=sT[:],
                                                identity=ident[:])
                            row = hp.tile([128, D], f32, tag="row")
                            if k == 1:
                                nc.vector.tensor_copy(row[:], ps2[:])
                            else:
                                tp = hp.tile([128, D], f32, tag="tp")
                                nc.sync.dma_start(
                                    tp[:],
                                    t_own[k - 2][blk * 128:(blk + 1) * 128, :])
                                nc.vector.tensor_tensor(
                                    out=row[:], in0=ps2[:], in1=tp[:],
                                    op=mybir.AluOpType.subtract)
                            nc.sync.dma_start(
                                t_own[k][blk * 128:(blk + 1) * 128, :], row[:])
                        if k < K:
                            done = ((grp + 1) * BPC * 128) // PART_PAD
                            while parts_done < done:
                                q = parts_done
                                nc.gpsimd.collective_compute(
                                    "AllGather", mybir.AluOpType.bypass,
                                    replica_groups=rg,
                                    ins=[t_own[k][q * PART_PAD:(q + 1) * PART_PAD, :]],
                                    outs=[t_full[k % 2][q][:]])
                                parts_done += 1

            # ---------------- z = sum gamma_k T_k ----------------
            with tc.tile_pool(name="z", bufs=2) as zp, \
                 tc.tile_pool(name="zc", bufs=1) as zc:
                zacc = zc.tile([128, FLAT], f32)
                for k in range(K + 1):
                    tt = zp.tile([128, FLAT], f32, tag="zt")
                    nc.sync.dma_start(
                        tt[:], t_own[k][:].rearrange("(p n) d -> p (n d)", p=128))
                    if k == 0:
                        nc.vector.tensor_scalar_mul(zacc[:], tt[:],
                                                    float(gamma[0]))
                    else:
                        tmp = zp.tile([128, FLAT], f32, tag="ztmp")
                        nc.vector.tensor_scalar_mul(tmp[:], tt[:],
                                                    float(gamma[k]))
                        nc.vector.tensor_tensor(out=zacc[:], in0=zacc[:],
                                                in1=tmp[:],
                                                op=mybir.AluOpType.add)
                nc.sync.dma_start(z_out[:], zacc[:])

    nc.compile()
    return nc


_CACHE = {}


def kernel(x, edge_row, edge_col, edge_vals, W1, b1, W2, b2, gamma,
           _cfg=None, _run=True):
    from concourse import bass_utils

    c = _derived(_cfg or FULL)
    x = np.ascontiguousarray(np.asarray(x, np.float32))
    W1 = np.ascontiguousarray(np.asarray(W1, np.float32))
    W2 = np.ascontiguousarray(np.asarray(W2, np.float32))
    b1 = np.asarray(b1, np.float32)
    b2 = np.asarray(b2, np.float32)
    gamma = np.asarray(gamma, np.float32)

    prep = _preprocess(edge_row, edge_col, edge_vals, c)

    key = (gamma.tobytes(), tuple(sorted(c.items())))
    if key not in _CACHE:
        _CACHE[key] = _build_program(c, gamma)
    nc = _CACHE[key]

    NCc, RPC, R = c["NC"], c["RPC"], c["R"]
    b1p = np.ascontiguousarray(b1.reshape(HID // 128, 128).T)
    b2p = np.ascontiguousarray(b2.reshape(D, 1))
    in_maps = []
    for m in range(NCc):
        pm = prep["perm"][m]
        xp = np.zeros((R, IN_DIM), np.float32)
        valid = pm >= 0
        xp[valid] = x[m * RPC + pm[valid]]
        cm = prep["cores"][m]
        in_maps.append({
            "xT": np.ascontiguousarray(xp.T),
            "W1": W1, "W2": W2, "b1p": b1p, "b2p": b2p,
            "idxT": cm["idx16"], "sel1": cm["sel1"], "sel2": cm["sel2"],
        })
    if not _run:
        return nc, in_maps, prep, c

    res = bass_utils.run_bass_kernel_spmd(
        nc, in_maps, core_ids=list(range(NCc)), trace=False)

    z = np.zeros((c["N"], D), np.float32)
    for m in range(NCc):
        zc = res.results[m]["z_out"].reshape(R, D)
        pm = prep["perm"][m]
        valid = pm >= 0
        z[m * RPC + pm[valid]] = zc[valid]
    return z


# revision 7
# speedup vs baseline: 1.2490x; 1.2490x over previous
"""ChebNet (gnn_message_passing) Trainium2 kernel — 8 NeuronCores, SPMD.

Strategy:
- Nodes row-sharded 12500/core (edge_row is sorted). Each core's rows are
  pre-assigned to 4 fixed "parts" (by original index), padded to 3200 rows,
  and permuted within-part by a window-packing permutation.
- Per Chebyshev hop: AllGather of the [3200,64] part tables builds each
  core's full gather table [102400, 64]; the 4 parts double as the int16
  source buckets for dma_gather AND as the 4 chunked AllGathers.
- spmm = dma_gather (256B rows) + selector matmuls: chunk = 128 gathered
  rows (lhsT [128 slots, 64]) x selector rhs [128, 13] -> PSUM [64, 13-row
  window slice], accumulated across the 4 buckets; transposed back to
  row-major via TensorE; T_{k+1} = 2*S - T_{k-1} with the 2x folded into
  the hop>=2 selector tables and T_{k-1} streamed from DRAM.
- Encoder MLP runs transposed (h1T = W1c.T @ xT) so biases are
  per-partition; x is transposed/permuted on host.
- z = sum_k gamma_k T_k computed in a final batched pass over the 9 stored
  T tables.
"""
import numpy as np

# ---------------- configuration (hardcoded for this problem) ----------------
IN_DIM, HID, D = 512, 256, 64
KHOPS = 8
BLK = 128
W = 13                      # window width (psum free-dim slice per chunk)
NW = 10                     # chunks per (block, bucket)
WLAST = BLK - W * (NW - 1)  # 11

FULL = dict(N=100000, NC=8, NPART=4, PART_REAL=3125, PART_PAD=3200, BPC=5,
            KHOPS=8)


def _derived(cfg):
    c = dict(cfg)
    c["RPC"] = c["N"] // c["NC"]
    c["R"] = c["NPART"] * c["PART_PAD"]
    c["NBLK"] = c["R"] // BLK
    c["BUCKET"] = c["NC"] * c["PART_PAD"]
    c["G"] = c["NPART"] * c["BUCKET"]
    c["NGRP"] = c["NBLK"] // c["BPC"]
    c["IPC"] = c["BPC"] * NW * 128          # idxs per gather call
    return c


# ---------------------------- host preprocessing ----------------------------

def _pack_windows(deg_vec, part_pad):
    """deg_vec [PART_REAL, 4] -> within-part permuted position for each row,
    s.t. every (13-row window, bucket) degree sum <= 128."""
    nwin = (part_pad // BLK) * NW
    win_rows = np.array([WLAST if (w % NW) == NW - 1 else W for w in range(nwin)])
    loads = np.zeros((nwin, 4), np.int64)
    used = np.zeros(nwin, np.int64)
    order = np.argsort(-deg_vec.sum(1), kind="stable")
    win_of = np.empty(len(deg_vec), np.int64)
    for r in order:
        d = deg_vec[r]
        ok = (used < win_rows) & ((loads + d) <= 128).all(1)
        if not ok.any():
            raise RuntimeError("window packing infeasible; increase NW")
        cand = np.where(ok)[0]
        w = cand[np.argmin((loads[cand] + d).max(1))]
        win_of[r] = w
        loads[w] += d
        used[w] += 1
    j = np.zeros(len(deg_vec), np.int64)
    fill = np.zeros(nwin, np.int64)
    for r in order:
        w = win_of[r]
        j[r] = (w // NW) * BLK + (w % NW) * W + fill[w]
        fill[w] += 1
    return j


def _preprocess(edge_row, edge_col, edge_vals, c):
    NC, RPC, PART_REAL = c["NC"], c["RPC"], c["PART_REAL"]
    PART_PAD, R, NBLK = c["PART_PAD"], c["R"], c["NBLK"]
    BUCKET, NGRP, BPC, IPC = c["BUCKET"], c["NGRP"], c["BPC"], c["IPC"]
    N = c["N"]

    edge_row = np.asarray(edge_row).astype(np.int64)
    edge_col = np.asarray(edge_col).astype(np.int64)
    edge_vals = np.asarray(edge_vals).astype(np.float32)
    src_part = (edge_col % RPC) // PART_REAL

    lo = np.searchsorted(edge_row, np.arange(NC) * RPC)
    hi = np.searchsorted(edge_row, (np.arange(NC) + 1) * RPC)
    j_all = np.zeros((NC, c["NPART"], PART_REAL), np.int64)
    for m in range(NC):
        rl = edge_row[lo[m]:hi[m]] - m * RPC
        q = src_part[lo[m]:hi[m]]
        dv = np.zeros((RPC, 4), np.int64)
        np.add.at(dv, (rl, q), 1)
        for p in range(c["NPART"]):
            j_all[m, p] = _pack_windows(dv[p * PART_REAL:(p + 1) * PART_REAL],
                                        PART_PAD)

    v = np.arange(N)
    m_of, local = v // RPC, v % RPC
    p_of, o_of = local // PART_REAL, local % PART_REAL
    pos = p_of * BUCKET + m_of * PART_PAD + j_all[m_of, p_of, o_of]

    perm = np.full((NC, R), -1, np.int64)
    r_in_core = p_of * PART_PAD + j_all[m_of, p_of, o_of]
    perm[m_of, r_in_core] = local

    cores = []
    for m in range(NC):
        rl = edge_row[lo[m]:hi[m]] - m * RPC
        q = src_part[lo[m]:hi[m]]
        vals = edge_vals[lo[m]:hi[m]]
        boff = pos[edge_col[lo[m]:hi[m]]] - q * BUCKET
        gl = m * RPC + rl
        rr_core = p_of[gl] * PART_PAD + j_all[m, p_of[gl], o_of[gl]]
        b = rr_core // BLK
        rr = rr_core % BLK
        wi = np.minimum(rr // W, NW - 1)
        row_in_win = rr - wi * W

        chunk = (q * NBLK + b) * NW + wi          # bucket-major global chunk
        order = np.argsort(chunk, kind="stable")
        cs = chunk[order]
        slot = np.arange(len(cs)) - np.searchsorted(cs, cs)
        assert slot.max() < 128, "chunk overflow"

        nch = 4 * NBLK * NW
        idx_full = np.zeros((nch, 128), np.int64)
        sel_full = np.zeros((nch, 128, W), np.float32)
        idx_full[cs, slot] = boff[order]
        sel_full[cs, slot, row_in_win[order]] = vals[order]
        nfill = np.zeros(nch, np.int64)
        np.add.at(nfill, cs, 1)
        padm = np.arange(128)[None, :] >= nfill[:, None]
        idx_full[padm] = np.broadcast_to(idx_full[:, :1], idx_full.shape)[padm]

        # regroup chunks: device order is (q, grp, bl, wi); chunk id above is
        # (q, blk, wi) with blk = grp*BPC+bl -> already linear in that order.
        ncalls = 4 * NGRP
        idx_call = idx_full.reshape(ncalls, IPC)
        assert idx_call.max() < 2 ** 15
        i16 = idx_call.reshape(ncalls, IPC // 16, 16)
        idx16 = np.tile(i16.transpose(0, 2, 1), (1, 8, 1))  # [ncalls,128,IPC//16]
        idx16 = np.ascontiguousarray(idx16).astype(np.int16)

        sel = sel_full.reshape(ncalls, BPC * NW, 128, W)
        sel = np.ascontiguousarray(sel.transpose(0, 2, 1, 3)).reshape(
            ncalls, 128, BPC * NW * W)
        cores.append({"idx16": idx16.reshape(4, NGRP, 128, IPC // 16),
                      "sel1": sel.reshape(4, NGRP, 128, BPC * NW * W),
                      "sel2": (2.0 * sel).reshape(4, NGRP, 128, BPC * NW * W)})
    return {"pos": pos, "perm": perm, "cores": cores}


# ------------------------------ device program ------------------------------

def _build_program(c, gamma):
    import concourse.bacc as bacc
    import concourse.mybir as mybir
    from concourse.tile import TileContext
    from concourse.masks import make_identity

    f32, i16 = mybir.dt.float32, mybir.dt.int16
    NPART, R, NBLK, BUCKET, G = c["NPART"], c["R"], c["NBLK"], c["BUCKET"], c["G"]
    NGRP, BPC, IPC, K = c["NGRP"], c["BPC"], c["IPC"], c["KHOPS"]
    PART_PAD = c["PART_PAD"]
    RT = 512                               # MLP row-tile
    NRT = R // RT
    FLAT = R * D // 128                    # flat free-dim per partition

    nc = bacc.Bacc("TRN2", target_bir_lowering=False, num_devices=c["NC"])
    rg = [list(range(c["NC"]))]

    xT = nc.dram_tensor("xT", [IN_DIM, R], f32, kind="ExternalInput")
    W1 = nc.dram_tensor("W1", [IN_DIM, HID], f32, kind="ExternalInput")
    W2 = nc.dram_tensor("W2", [HID, D], f32, kind="ExternalInput")
    b1p = nc.dram_tensor("b1p", [128, HID // 128], f32, kind="ExternalInput")
    b2p = nc.dram_tensor("b2p", [D, 1], f32, kind="ExternalInput")
    idxT = nc.dram_tensor("idxT", [4, NGRP, 128, IPC // 16], i16, kind="ExternalInput")
    sel1 = nc.dram_tensor("sel1", [4, NGRP, 128, BPC * NW * W], f32, kind="ExternalInput")
    sel2 = nc.dram_tensor("sel2", [4, NGRP, 128, BPC * NW * W], f32, kind="ExternalInput")
    z_out = nc.dram_tensor("z_out", [128, FLAT], f32, kind="ExternalOutput")

    with TileContext(nc) as tc:
        with tc.tile_pool(name="dram", bufs=1, space="DRAM") as dp, \
             tc.tile_pool(name="const", bufs=1) as cp:
            t_own = [dp.tile([R, D], f32, tag=f"town{k}", name=f"town{k}")
                     for k in range(K + 1)]
            t_full = [[dp.tile([BUCKET, D], f32, tag=f"tf{i}_{q}",
                               name=f"tf{i}_{q}")
                       for q in range(4)] for i in range(2)]
            ident = cp.tile([64, 64], f32)
            make_identity(nc, ident)

            # ---------------- MLP (transposed) ----------------
            with tc.tile_pool(name="mlp", bufs=2) as mp, \
                 tc.tile_pool(name="mcp", bufs=1) as mcp, \
                 tc.tile_pool(name="mpp", bufs=2, space="PSUM") as mpp, \
                 tc.tile_pool(name="mpp2", bufs=2, space="PSUM") as mpp2:
                w1_sb = mcp.tile([128, (IN_DIM // 128) * HID], f32)
                for kc in range(IN_DIM // 128):
                    nc.sync.dma_start(w1_sb[:, kc * HID:(kc + 1) * HID],
                                      W1[kc * 128:(kc + 1) * 128, :])
                w2_sb = mcp.tile([128, (HID // 128) * D], f32)
                for kc in range(HID // 128):
                    nc.sync.dma_start(w2_sb[:, kc * D:(kc + 1) * D],
                                      W2[kc * 128:(kc + 1) * 128, :])
                b1_sb = mcp.tile([128, HID // 128], f32)
                nc.sync.dma_start(b1_sb[:], b1p[:])
                b2_sb = mcp.tile([D, 1], f32)
                nc.sync.dma_start(b2_sb[:], b2p[:])

                parts_done = 0
                for rt in range(NRT):
                    xt = mp.tile([128, (IN_DIM // 128) * RT], f32, tag="xt")
                    for kc in range(IN_DIM // 128):
                        nc.sync.dma_start(
                            xt[:, kc * RT:(kc + 1) * RT],
                            xT[kc * 128:(kc + 1) * 128, rt * RT:(rt + 1) * RT])
                    h1 = mp.tile([128, (HID // 128) * RT], f32, tag="h1")
                    for h in range(HID // 128):
                        ps1 = mpp.tile([128, RT], f32, space="PSUM", tag="ps1")
                        for kc in range(IN_DIM // 128):
                            nc.tensor.matmul(
                                ps1[:],
                                lhsT=w1_sb[:, kc * HID + h * 128:
                                           kc * HID + (h + 1) * 128],
                                rhs=xt[:, kc * RT:(kc + 1) * RT],
                                start=(kc == 0), stop=(kc == IN_DIM // 128 - 1))
                        nc.scalar.activation(
                            out=h1[:, h * RT:(h + 1) * RT], in_=ps1[:],
                            func=mybir.ActivationFunctionType.Relu,
                            bias=b1_sb[:, h:h + 1], scale=1.0)
                    ps2 = mpp.tile([D, RT], f32, space="PSUM", tag="ps2")
                    for kc in range(HID // 128):
                        nc.tensor.matmul(ps2[:],
                                         lhsT=w2_sb[:, kc * D:(kc + 1) * D],
                                         rhs=h1[:, kc * RT:(kc + 1) * RT],
                                         start=(kc == 0),
                                         stop=(kc == HID // 128 - 1))
                    hT = mp.tile([D, RT], f32, tag="hT")
                    nc.scalar.activation(out=hT[:], in_=ps2[:],
                                         func=mybir.ActivationFunctionType.Identity,
                                         bias=b2_sb[:, 0:1], scale=1.0)
                    for sb in range(RT // 128):
                        ps3 = mpp2.tile([128, D], f32, space="PSUM", tag="ps3")
                        nc.tensor.transpose(out=ps3[:],
                                            in_=hT[:, sb * 128:(sb + 1) * 128],
                                            identity=ident[:])
                        row = mp.tile([128, D], f32, tag="mrow")
                        nc.scalar.copy(out=row[:], in_=ps3[:])
                        blk = rt * (RT // 128) + sb
                        nc.sync.dma_start(
                            t_own[0][blk * 128:(blk + 1) * 128, :], row[:])
                    done = ((rt + 1) * RT) // PART_PAD
                    while parts_done < done:
                        q = parts_done
                        nc.gpsimd.collective_compute(
                            "AllGather", mybir.AluOpType.bypass,
                            replica_groups=rg,
                            ins=[t_own[0][q * PART_PAD:(q + 1) * PART_PAD, :]],
                            outs=[t_full[0][q][:]])  # hop-0 tables in set 0
                        parts_done += 1

            # ---------------- Chebyshev hops ----------------
            for k in range(1, K + 1):
                seld = sel1 if k == 1 else sel2
                src = t_full[(k - 1) % 2]
                with tc.tile_pool(name=f"h{k}", bufs=2) as hp, \
                     tc.tile_pool(name=f"hpp{k}", bufs=3, space="PSUM") as hpp, \
                     tc.tile_pool(name=f"hpp2{k}", bufs=3, space="PSUM") as hpp2:
                    parts_done = 0
                    for grp in range(NGRP):
                        gts, sts = [], []
                        for q in range(4):
                            it = hp.tile([128, IPC // 16], i16, tag=f"idx{q}")
                            nc.sync.dma_start(it[:], idxT[q, grp])
                            st = hp.tile([128, BPC * NW * W], f32, tag=f"sel{q}")
                            nc.sync.dma_start(st[:], seld[q, grp])
                            gt = hp.tile([128, BPC * NW * D], f32, tag=f"g{q}")
                            nc.gpsimd.dma_gather(
                                out_ap=gt[:].rearrange("p (c d) -> p c d", d=D),
                                in_ap=src[q][:],
                                idxs_ap=it[:],
                                num_idxs=IPC, num_idxs_reg=IPC, elem_size=D,
                                single_packet=False)
                            gts.append(gt)
                            sts.append(st)
                        for bl in range(BPC):
                            blk = grp * BPC + bl
                            ps = hpp.tile([64, 128], f32, space="PSUM", tag="ps")
                            for wi in range(NW):
                                lc = bl * NW + wi
                                n = WLAST if wi == NW - 1 else W
                                for q in range(4):
                                    nc.tensor.matmul(
                                        ps[:, wi * W:wi * W + n],
                                        lhsT=gts[q][:, lc * D:(lc + 1) * D],
                                        rhs=sts[q][:, lc * W:lc * W + n],
                                        start=(q == 0), stop=(q == 3))
                            sT = hp.tile([64, 128], f32, tag="sT")
                            nc.scalar.copy(out=sT[:], in_=ps[:])
                            ps2 = hpp2.tile([128, D], f32, space="PSUM", tag="ps2")
                            nc.tensor.transpose(out=ps2[:], in_=sT[:],
                                                identity=ident[:])
                            row = hp.tile([128, D], f32, tag="row")
                            if k == 1:
                                nc.vector.tensor_copy(row[:], ps2[:])
                            else:
                                tp = hp.tile([128, D], f32, tag="tp")
                                nc.sync.dma_start(
                                    tp[:],
                                    t_own[k - 2][blk * 128:(blk + 1) * 128, :])
                                nc.vector.tensor_tensor(
                                    out=row[:], in0=ps2[:], in1=tp[:],
                                    op=mybir.AluOpType.subtract)
                            nc.sync.dma_start(
                                t_own[k][blk * 128:(blk + 1) * 128, :], row[:])
                        if k < K:
                            done = ((grp + 1) * BPC * 128) // PART_PAD
                            while parts_done < done:
                                q = parts_done
                                nc.gpsimd.collective_compute(
                                    "AllGather", mybir.AluOpType.bypass,
                                    replica_groups=rg,
                                    ins=[t_own[k][q * PART_PAD:(q + 1) * PART_PAD, :]],
                                    outs=[t_full[k % 2][q][:]])
                                parts_done += 1

            # ---------------- z = sum gamma_k T_k ----------------
            with tc.tile_pool(name="z", bufs=2) as zp, \
                 tc.tile_pool(name="zc", bufs=1) as zc:
                zacc = zc.tile([128, FLAT], f32)
                for k in range(K + 1):
                    tt = zp.tile([128, FLAT], f32, tag="zt")
                    nc.sync.dma_start(
                        tt[:], t_own[k][:].rearrange("(p n) d -> p (n d)", p=128))
                    if k == 0:
                        nc.vector.tensor_scalar_mul(zacc[:], tt[:],
                                                    float(gamma[0]))
                    else:
                        tmp = zp.tile([128, FLAT], f32, tag="ztmp")
                        nc.vector.tensor_scalar_mul(tmp[:], tt[:],
                                                    float(gamma[k]))
                        nc.vector.tensor_tensor(out=zacc[:], in0=zacc[:],
                                                in1=tmp[:],
                                                op=mybir.AluOpType.add)
                nc.sync.dma_start(z_out[:], zacc[:])

    nc.compile()
    return nc


_CACHE = {}
LAST_SPMD_WALL = None


def kernel(x, edge_row, edge_col, edge_vals, W1, b1, W2, b2, gamma,
           _cfg=None, _run=True):
    from concourse import bass_utils

    c = _derived(_cfg or FULL)
    x = np.ascontiguousarray(np.asarray(x, np.float32))
    W1 = np.ascontiguousarray(np.asarray(W1, np.float32))
    W2 = np.ascontiguousarray(np.asarray(W2, np.float32))
    b1 = np.asarray(b1, np.float32)
    b2 = np.asarray(b2, np.float32)
    gamma = np.asarray(gamma, np.float32)

    prep = _preprocess(edge_row, edge_col, edge_vals, c)

    key = (gamma.tobytes(), tuple(sorted(c.items())))
    if key not in _CACHE:
        _CACHE[key] = _build_program(c, gamma)
    nc = _CACHE[key]

    NCc, RPC, R = c["NC"], c["RPC"], c["R"]
    b1p = np.ascontiguousarray(b1.reshape(HID // 128, 128).T)
    b2p = np.ascontiguousarray(b2.reshape(D, 1))
    in_maps = []
    for m in range(NCc):
        pm = prep["perm"][m]
        xp = np.zeros((R, IN_DIM), np.float32)
        valid = pm >= 0
        xp[valid] = x[m * RPC + pm[valid]]
        cm = prep["cores"][m]
        in_maps.append({
            "xT": np.ascontiguousarray(xp.T),
            "W1": W1, "W2": W2, "b1p": b1p, "b2p": b2p,
            "idxT": cm["idx16"], "sel1": cm["sel1"], "sel2": cm["sel2"],
        })
    if not _run:
        return nc, in_maps, prep, c

    import time as _time
    _t0 = _time.time()
    res = bass_utils.run_bass_kernel_spmd(
        nc, in_maps, core_ids=list(range(NCc)), trace=False)
    global LAST_SPMD_WALL
    LAST_SPMD_WALL = _time.time() - _t0

    z = np.zeros((c["N"], D), np.float32)
    for m in range(NCc):
        zc = res.results[m]["z_out"].reshape(R, D)
        pm = prep["perm"][m]
        valid = pm >= 0
        z[m * RPC + pm[valid]] = zc[valid]
    return z
